# revision 1
# baseline (speedup 1.0000x reference)
"""Causal self-attention (B=4, S=2048, D=1024, H=16) on 8 Trainium2 NeuronCores.

Sharding: 8 cores = 4 batches x 2 head-groups (8 heads each).
Per core: QKV projections, flash-style causal attention with scores computed
transposed ([k, q] layout), exp on ScalarE (no max subtraction -- scores are
O(1) here), softmax denominator via an appended ones-column in the attn@V
matmul, out-projection against a W_O column slice.  The inter-core
"all-reduce" for the out-projection (row-parallel W_O) is a host-side sum of
the two head-group partials per batch.

All matmuls are bf16 (fp32 PSUM accumulation).  Causal structure is exploited
exactly on the diagonal 128-key chunks: the scores matmul and the attn@V
matmul are restricted to the valid query window [128*jj, 512), the exp of the
second diagonal chunk-pair starts at column 256, and a single shared
[128,128] lower-triangle mask zeroes the intra-chunk upper triangle (the
region below each chunk's window is never read by the restricted attn@V).

The attention inner loop is Scalar-bound (2 exps ~2.1us vs ~1.7us of PE work
per chunk pair), so projection matmul groups for the NEXT head-pair block are
interleaved one-per-iteration into the attention stream as PE filler.
"""

import os
import sys

for _p in ("/opt/trn_rl_repo",):
    if _p not in sys.path and os.path.isdir(_p):
        sys.path.insert(0, _p)

import numpy as np

B, S, D, H, DK = 4, 2048, 1024, 16, 64
N_CORES = 8
EC = 512          # e-dims (= head-dim columns) per core: 8 heads x 64
N_D = D // 128    # 8 contraction chunks for projections
N_SC = S // 128   # 16 key chunks
N_QB = S // 512   # 4 query blocks

_CACHE = {}


def _build():
    import concourse.mybir as mybir
    import concourse.tile as tile
    from concourse import bacc
    from contextlib import ExitStack

    fp32 = mybir.dt.float32
    bf16 = mybir.dt.bfloat16
    AF = mybir.ActivationFunctionType
    Alu = mybir.AluOpType

    nc = bacc.Bacc(trn_type="TRN2", target_bir_lowering=False, debug=False)

    xt_d = nc.dram_tensor("xt", [D, S], bf16, kind="ExternalInput")
    wq_d = nc.dram_tensor("wqt", [D, EC], bf16, kind="ExternalInput")
    wk_d = nc.dram_tensor("wkt", [D, EC], bf16, kind="ExternalInput")
    wv_d = nc.dram_tensor("wvt", [D, EC], bf16, kind="ExternalInput")
    wo_d = nc.dram_tensor("wot", [EC, D], bf16, kind="ExternalInput")
    yt_d = nc.dram_tensor("yt", [D, S], fp32, kind="ExternalOutput")

    with tile.TileContext(nc) as tc, ExitStack() as ctx:
        # ---- persistent results of phase 1 ------------------------------
        proj_out_pool = ctx.enter_context(tc.tile_pool(name="projout", bufs=1))
        qt_sb = [proj_out_pool.tile([128, S], bf16, name=f"qt{ec}", tag=f"qt{ec}") for ec in range(4)]
        kt_sb = [proj_out_pool.tile([128, S], bf16, name=f"kt{ec}", tag=f"kt{ec}") for ec in range(4)]
        # v_sb[sc]: per head h a 128-col stationary block:
        #   even h: [V(64) | ones(col 64) | unused(63)] -> psum rows 0..64
        #   odd  h: [zeros(0:63), ones at col 32 | V(64) at 64:128]
        #           -> psum row 32 = n, rows 64..127 = out
        v_sb = [proj_out_pool.tile([128, 8, 128], bf16, name=f"v{sc}", tag=f"v{sc}") for sc in range(N_SC)]

        # ---- input tiles (all bf16) -------------------------------------
        xw_pool = ctx.enter_context(tc.tile_pool(name="xw", bufs=1))
        xt_sb, wq_sb, wk_sb, wv_sb = [], [], [], []
        for d in range(N_D):
            t = xw_pool.tile([128, S], bf16, name=f"x{d}", tag=f"x{d}")
            nc.sync.dma_start(t[:], xt_d.ap()[128 * d:128 * (d + 1), :])
            xt_sb.append(t)
        for lst, dram, nm in ((wq_sb, wq_d, "q"), (wk_sb, wk_d, "k"), (wv_sb, wv_d, "v")):
            for d in range(N_D):
                t = xw_pool.tile([128, EC], bf16, name=f"w{nm}{d}", tag=f"w{nm}{d}")
                nc.gpsimd.dma_start(t[:], dram.ap()[128 * d:128 * (d + 1), :])
                lst.append(t)
        wo_sb = []
        for cc in range(4):
            t = xw_pool.tile([128, D], bf16, name=f"wo{cc}", tag=f"wo{cc}")
            nc.gpsimd.dma_start(t[:], wo_d.ap()[128 * cc:128 * (cc + 1), :])
            wo_sb.append(t)

        # ---- constant lower-triangle mask (keep where q_local >= k_row) --
        const_pool = ctx.enter_context(tc.tile_pool(name="const", bufs=1))
        tri = const_pool.tile([128, 128], bf16, name="tri", tag="tri")
        nc.gpsimd.memset(tri[:], 1.0)
        nc.gpsimd.affine_select(
            out=tri[:], in_=tri[:], compare_op=Alu.is_ge, fill=0.0,
            base=0, pattern=[[1, 128]], channel_multiplier=-1,
        )

        dram_pool = ctx.enter_context(tc.tile_pool(name="drs", bufs=4, space="DRAM"))
        ps_score = ctx.enter_context(tc.tile_pool(name="psscore", bufs=2, space="PSUM"))
        ps_av = ctx.enter_context(tc.tile_pool(name="psav", bufs=1, space="PSUM"))
        ps_proj = tc.alloc_tile_pool(name="psproj", bufs=2, space="PSUM")
        attn_pool = ctx.enter_context(tc.tile_pool(name="attn", bufs=7))
        rb_pool = ctx.enter_context(tc.tile_pool(name="rb", bufs=3))
        ou_pool = ctx.enter_context(tc.tile_pool(name="ou", bufs=3))
        outn_pool = ctx.enter_context(tc.tile_pool(name="outn", bufs=4))
        y_pool = ctx.enter_context(tc.tile_pool(name="ysb", bufs=3))

        def qk_proj_group(ec, sb_, which):
            w_sb, out_sb = (wq_sb, qt_sb) if which == "q" else (wk_sb, kt_sb)
            ps = ps_proj.tile([128, 512], fp32, name="pp", tag="pp")
            for d in range(N_D):
                nc.tensor.matmul(
                    ps[:],
                    w_sb[d][:, 128 * ec:128 * (ec + 1)],
                    xt_sb[d][:, 512 * sb_:512 * (sb_ + 1)],
                    start=(d == 0), stop=(d == N_D - 1),
                )
            nc.vector.tensor_copy(out_sb[ec][:, 512 * sb_:512 * (sb_ + 1)], ps[:])

        def emit_v_proj(sc):
            ps = ps_proj.tile([128, 512], fp32, name="pv", tag="pp")
            for d in range(N_D):
                nc.tensor.matmul(
                    ps[:],
                    xt_sb[d][:, 128 * sc:128 * (sc + 1)],
                    wv_sb[d][:],
                    start=(d == 0), stop=(d == N_D - 1),
                )
            vt = v_sb[sc]
            for h in range(8):
                if h % 2 == 0:
                    nc.vector.tensor_copy(vt[:, h, 0:64], ps[:, 64 * h:64 * h + 64])
                    nc.gpsimd.memset(vt[:, h, 64:65], 1.0)
                else:
                    nc.gpsimd.memset(vt[:, h, 0:63], 0.0)
                    nc.gpsimd.memset(vt[:, h, 32:33], 1.0)
                    nc.vector.tensor_copy(vt[:, h, 64:128], ps[:, 64 * h:64 * h + 64])

        def proj_block_groups(k):
            gs = [(lambda ec, sb_, w: (lambda: qk_proj_group(ec, sb_, w)))(k, sb_, w)
                  for w in ("q", "k") for sb_ in range(4)]
            gs += [(lambda sc: (lambda: emit_v_proj(sc)))(sc)
                   for sc in range(4 * k, 4 * k + 4)]
            return gs

        def emit_attn(qb, hp, outn, filler=None):
            hA, hB = 2 * hp, 2 * hp + 1
            qt, kt = qt_sb[hp], kt_sb[hp]
            nkc = 4 * qb + 4

            def win(kc):
                jj = kc - (nkc - 4)
                return 128 * jj if jj > 0 else 0

            def av_mms(ps_o, h, half, at, kc):
                m_sz = 65 if h % 2 == 0 else 128
                w0 = win(kc)
                nc.tensor.matmul(
                    ps_o[0:m_sz, w0:512],
                    v_sb[kc][:, h, 0:m_sz],
                    at[:, half, w0:512],
                    start=(kc == 0), stop=(kc == nkc - 1),
                    skip_group_check=True,
                )

            ps_oA = ps_av.tile([128, 512], fp32, name="poA", tag="poA")
            ps_oB = ps_av.tile([128, 512], fp32, name="poB", tag="poB")
            pend = []
            # per key chunk: one [128, 2, 512] score tile holding both
            # head-halves; one strided exp covers both at the exact window;
            # attn@V lags 3 chunks so the exp->mask chain never gates PE
            for kc in range(nkc):
                w0 = win(kc)
                ps_s = ps_score.tile([128, 2, 512], fp32, name="ps", tag="ps")
                nc.tensor.matmul(
                    ps_s[:, 0, w0:512],
                    kt[0:64, 128 * kc:128 * (kc + 1)],
                    qt[0:64, 512 * qb + w0:512 * (qb + 1)],
                    start=True, stop=True,
                )
                nc.tensor.matmul(
                    ps_s[:, 1, w0:512],
                    kt[64:128, 128 * kc:128 * (kc + 1)],
                    qt[64:128, 512 * qb + w0:512 * (qb + 1)],
                    start=True, stop=True,
                )
                at = attn_pool.tile([128, 2, 512], bf16, name="at", tag="at")
                nc.scalar.activation(at[:, :, w0:512], ps_s[:, :, w0:512], AF.Exp, scale=0.125)
                if kc >= nkc - 4:
                    nc.vector.tensor_mul(at[:, 0, w0:w0 + 128], at[:, 0, w0:w0 + 128], tri[:])
                    nc.vector.tensor_mul(at[:, 1, w0:w0 + 128], at[:, 1, w0:w0 + 128], tri[:])
                if len(pend) >= 3:
                    p = pend.pop(0)
                    av_mms(ps_oA, hA, 0, p[0], p[1])
                    av_mms(ps_oB, hB, 1, p[0], p[1])
                pend.append((at, kc))
                if filler is not None and kc % 2 == 1:
                    g = next(filler, None)
                    if g is not None:
                        g()
            for p in pend:
                av_mms(ps_oA, hA, 0, p[0], p[1])
                av_mms(ps_oB, hB, 1, p[0], p[1])

            # normalization: copy out + n rows off PSUM (frees banks),
            # broadcast both n rows into one base-0 tile via DRAM, one
            # full-tile fast reciprocal (base-0 only!), vector muls.
            ou = ou_pool.tile([128, 512], fp32, name="ou", tag="ou")
            rbn = rb_pool.tile([128, 512], fp32, name="rbn", tag="rbn")
            rbi = rb_pool.tile([128, 512], fp32, name="rbi", tag="rbi")
            rbb = rb_pool.tile([128, 512], fp32, name="rbb", tag="rbb")
            nc.vector.tensor_copy(ou[0:64, :], ps_oA[0:64, :])
            nc.vector.tensor_copy(rbn[64:65, :], ps_oA[64:65, :])
            nc.vector.tensor_copy(ou[64:128, :], ps_oB[64:128, :])
            nc.vector.tensor_copy(rbn[32:33, :], ps_oB[32:33, :])
            rdA = dram_pool.tile([1, 512], fp32, name="rdA", tag="rdA")
            rdB = dram_pool.tile([1, 512], fp32, name="rdB", tag="rdB")
            nc.sync.dma_start(rdA[:], rbn[64:65, :])
            nc.sync.dma_start(rbb[0:64, :], rdA[0:1, :].to_broadcast((64, 512)))
            nc.sync.dma_start(rdB[:], rbn[32:33, :])
            nc.sync.dma_start(rbb[64:128, :], rdB[0:1, :].to_broadcast((64, 512)))
            nc.vector.reciprocal_approx_fast(out=rbi[:, :], in_=rbb[:, :])
            nc.vector.tensor_mul(outn[hp][0:64, :], ou[0:64, :], rbi[0:64, :])
            nc.vector.tensor_mul(outn[hp][64:128, :], ou[64:128, :], rbi[64:128, :])

        def outproj_group(qb, outn, dc):
            ps = ps_y.tile([128, 512], fp32, name="py", tag="py")
            for hp in range(4):
                nc.tensor.matmul(
                    ps[:],
                    wo_sb[hp][:, 128 * dc:128 * (dc + 1)],
                    outn[hp][:],
                    start=(hp == 0), stop=(hp == 3),
                )
            ysb = y_pool.tile([128, 512], fp32, name="y", tag="y")
            nc.vector.tensor_copy(ysb[:], ps[:])
            nc.sync.dma_start(
                yt_d.ap()[128 * dc:128 * (dc + 1), 512 * qb:512 * (qb + 1)],
                ysb[:])

        def outproj_groups(qb, outn):
            return [(lambda dc: (lambda: outproj_group(qb, outn, dc)))(dc)
                    for dc in range(8)]

        # ---- interleaved emission ---------------------------------------
        # attn(qb, hp) becomes emittable after proj block k = max(hp, qb).
        # Epoch k's attention stream carries proj block k+1 as PE filler.
        outn_all = {qb: [outn_pool.tile([128, 512], bf16, name=f"on{qb}{hp}", tag=f"on{hp}")
                         for hp in range(4)] for qb in range(N_QB)}
        for g in proj_block_groups(0):
            g()
        for k in range(3):
            filler = iter(proj_block_groups(k + 1))
            for qb in range(N_QB):
                for hp in range(4):
                    if max(hp, qb) == k:
                        emit_attn(qb, hp, outn_all[qb], filler)
            for g in filler:
                g()
        ps_proj.release()
        ps_y = ctx.enter_context(tc.tile_pool(name="psy", bufs=2, space="PSUM"))
        emit_attn(3, 3, outn_all[3])
        for qb in range(3):
            emit_attn(qb, 3, outn_all[qb])
            # outproj(qb) is ready once attn(qb,3) has normalized; its 8
            # dc-groups ride the big attn(3,qb) pair as PE filler
            f = iter(outproj_groups(qb, outn_all[qb]))
            emit_attn(3, qb, outn_all[3], f)
            for g in f:
                g()
        for g in outproj_groups(3, outn_all[3]):
            g()

    nc.compile()
    return nc


def _get_nc():
    if "nc" not in _CACHE:
        _CACHE["nc"] = _build()
    return _CACHE["nc"]


def _run(in_maps, trace=False, **kw):
    from concourse.bass_utils import run_bass_kernel_spmd
    nc = _get_nc()
    return run_bass_kernel_spmd(nc, in_maps, core_ids=list(range(N_CORES)),
                                trace=trace, **kw)


def _prep_inputs(x, W_Q, W_K, W_V, W_O):
    import ml_dtypes
    bf = ml_dtypes.bfloat16
    x = np.asarray(x, dtype=np.float32)
    W_Q = np.asarray(W_Q, dtype=np.float32)
    W_K = np.asarray(W_K, dtype=np.float32)
    W_V = np.asarray(W_V, dtype=np.float32)
    W_O = np.asarray(W_O, dtype=np.float32)
    in_maps = []
    for c in range(N_CORES):
        b, hg = divmod(c, 2)
        es = EC * hg
        in_maps.append({
            "xt": np.ascontiguousarray(x[b].T).astype(bf),
            "wqt": np.ascontiguousarray(W_Q[es:es + EC, :].T).astype(bf),
            "wkt": np.ascontiguousarray(W_K[es:es + EC, :].T).astype(bf),
            "wvt": np.ascontiguousarray(W_V[es:es + EC, :].T).astype(bf),
            "wot": np.ascontiguousarray(W_O[:, es:es + EC].T).astype(bf),
        })
    return in_maps


def _gather(results):
    y = np.empty((B, S, D), dtype=np.float32)
    for b in range(B):
        yt = results[2 * b]["yt"].astype(np.float32) + results[2 * b + 1]["yt"].astype(np.float32)
        y[b] = yt.T
    return y


def kernel(x, W_Q, W_K, W_V, W_O):
    in_maps = _prep_inputs(x, W_Q, W_K, W_V, W_O)
    res = _run(in_maps, trace=False)
    return _gather(res.results)

